# revision 69
# baseline (speedup 1.0000x reference)
"""Multi-head attention + residual + LayerNorm on 8 Trainium2 NeuronCores.

Reference computation (B=2, S=2048, D=1024, H=16, HD=64):
    q,k,v = split_heads(x@Wq+bq), ...       # [B,H,S,HD]
    attn  = softmax(q k^T / sqrt(HD))
    out   = (attn v) merged -> [B,S,D] @ Wp + bp
    y     = LayerNorm(x + out) * gamma + beta

Sharding: 8 cores = 2 batches x 4 query-slices of 512 rows.
Each core computes QKV projections for its 512-row slice (x and all
weights in fp8e4, DoubleRow matmuls: two 128-row contraction chunks per
pass). K^T and V slices (both fp8e4) are AllGathered across the 4 cores
of the same batch in 8 per-head-pair pieces, each fired as soon as that
head pair's K/V is ready so attention pipelines behind the gathers. The
V payload travels p-major with softmax-sum `ones` columns pre-embedded,
so every V transfer is a contiguous DMA and the attn@V DoubleRow matmul
gets the denominator row for free. Each core then runs attention for all
16 heads restricted to its 512 queries, then fp8 DoubleRow projection +
residual + LayerNorm (f32).

Softmax: scores accumulate in f32 PSUM; per key chunk, head A's exp runs
on the scalar engine's exact ACT (fp8 out) while head B's runs
concurrently on the vector engine via a one-pass Schraudolph bit-trick
(f32 -> uint8 code with round+saturate, bitcast as fp8e4; negative codes
clamp to +0.0 so no NaNs). A uniform exp(-1.2) shift keeps fp8 in range
and cancels in the softmax normalization, which divides by the gathered
`ones`-row sums via reciprocal + gpsimd partition-broadcast.

Measured: ~273 us HW exec, rel err ~4.4e-3 vs the f32 reference.
"""

import os

import ml_dtypes
import numpy as np

import concourse.bacc as bacc
import concourse.tile as tile
from concourse import mybir
from concourse.bass_utils import run_bass_kernel_spmd

B, S, D, H, HD = 2, 2048, 1024, 16, 64
EPS = 1e-5
NCORES = 8
SL = S // 4          # 512 query rows per core
GROUPS = [[0, 1, 2, 3], [4, 5, 6, 7]]
BF = mybir.dt.bfloat16
F32 = mybir.dt.float32
FP8 = mybir.dt.float8e4
U8 = mybir.dt.uint8
I16 = mybir.dt.int16
Act = mybir.ActivationFunctionType
Alu = mybir.AluOpType
DR = mybir.MatmulPerfMode.DoubleRow

# exp(s*0.125 + BSH); uniform shift cancels in softmax, keeps fp8 in range
BSH = -1.2
LOG2E = 1.4426950408889634
# one-pass Schraudolph to fp8e4 codes: u8 = round(s*EC1 + EC2), bitcast e4m3
# (negatives saturate to 0 == fp8 +0.0, so no NaN poisoning)
EC1 = 8 * LOG2E * 0.125
EC2 = 56.0 + 8 * LOG2E * BSH + 0.34
# kc chunks handled by the scalar engine's exact ACT exp (rest: DVE trick)
SCALAR_KCS = {0, 2, 4, 6, 8, 10, 12, 14}

KB = 128 * SL // 2       # K^T block as bf16-slot count (fp8 bytes = 128*SL)
VB = 128 * 4 * 160 // 2  # V block slots: p-major [p, k4, 160] incl ones cols
KVB = KB + VB            # 73728


def build_program():
    nc = bacc.Bacc("TRN2", target_bir_lowering=False, debug=False,
                   num_devices=NCORES)

    # ---- I/O ----
    xT_d = nc.dram_tensor("xT", [D, SL], FP8, kind="ExternalInput")
    xq_d = nc.dram_tensor("xq", [SL, D], F32, kind="ExternalInput")
    wq_d = nc.dram_tensor("wq", [D, D], FP8, kind="ExternalInput")
    wk_d = nc.dram_tensor("wk", [D, D], FP8, kind="ExternalInput")
    wv_d = nc.dram_tensor("wv", [D, D], FP8, kind="ExternalInput")
    wp_d = nc.dram_tensor("wp", [D, D], FP8, kind="ExternalInput")
    bq_d = nc.dram_tensor("bq", [D], F32, kind="ExternalInput")
    bk_d = nc.dram_tensor("bk", [D], F32, kind="ExternalInput")
    bv_d = nc.dram_tensor("bv", [D], F32, kind="ExternalInput")
    bp_d = nc.dram_tensor("bp", [D], BF, kind="ExternalInput")
    gamma_d = nc.dram_tensor("gamma", [D], F32, kind="ExternalInput")
    beta_d = nc.dram_tensor("beta", [D], F32, kind="ExternalInput")
    y_d = nc.dram_tensor("y", [SL, D], F32, kind="ExternalOutput")

    import concourse.bass as bass

    def bcast_ap(dram_t, parts=128):
        # replicate a [D] dram vector across `parts` partitions
        return bass.AP(tensor=dram_t, offset=0, ap=[[0, parts], [1, D]])

    with tile.TileContext(nc) as tc:
        with (
            tc.tile_pool(name="persist", bufs=1) as persist,
            tc.tile_pool(name="dram", bufs=1, space="DRAM") as dram,
        ):
            # persistent tiles
            qt_sb = persist.tile([128, 8, SL], FP8)       # Q^T
            xq_sb = persist.tile([128, 4, D], F32)        # natural x slice
            wp_sb = persist.tile([128, 8, 2, 512], FP8)
            outT_sb = persist.tile([128, 8, SL], FP8)     # normalized attn out^T
            bv_bc = persist.tile([128, D], F32)
            gamma_bc = persist.tile([128, D], F32)
            beta_bc = persist.tile([128, D], F32)
            bq_sb = persist.tile([128, 8], F32)
            bk_sb = persist.tile([128, 8], F32)
            ones_sb = persist.tile([1, 128], BF)
            eps_sb = persist.tile([128, 1], F32)
            bsh_sb = persist.tile([128, 1], F32)
            # V tiles, fp8, interleaved kc pairs for DoubleRow:
            # [part=key-in-chunk, kc-pair, kc-parity, 160] with
            # cols 0:64 = V_A, 64 = ones, 80:144 = V_B, 144 = ones
            # (head B at 80 keeps the LDWEIGHTS base 16-byte aligned)
            vh_tiles = [persist.tile([128, 8, 2, 160], FP8, name=f"vh{i}")
                        for i in range(2)]

            # DRAM scratch per head pair: K^T bf16 (p-major, KB elems) then
            # V fp8 (s-major, VB bf16 slots) in one bf16 tensor so a single
            # AllGather moves both.
            kvb_hp = [dram.tile([1, KVB], BF, name=f"kvb{i}") for i in range(8)]
            kvg_hp = [dram.tile([4, KVB], BF, name=f"kvg{i}") for i in range(8)]
            dumb_in = dram.tile([1, 128], BF, name="dumb_in")
            dumb_out = dram.tile([4, 128], BF, name="dumb_out")
            nc.gpsimd.collective_compute(
                "AllGather", Alu.bypass, replica_groups=GROUPS,
                ins=[dumb_in[:].opt()], outs=[dumb_out[:].opt()])

            # small/early loads first (biases feed phase-1 epilogues)
            nc.sync.dma_start(bk_sb[:], bk_d.ap().rearrange("(co p) -> p co", p=128))
            nc.sync.dma_start(bv_bc[:], bcast_ap(bv_d))
            nc.sync.dma_start(bq_sb[:], bq_d.ap().rearrange("(co p) -> p co", p=128))
            nc.vector.memset(ones_sb[:], 1.0)
            nc.vector.memset(eps_sb[:], EPS)
            nc.vector.memset(bsh_sb[:], BSH)

            def kvb_kview(hp):
                return kvb_hp[hp][0, 0:KB].bitcast(FP8).rearrange(
                    "(p s) -> p s", p=128)

            def kvb_vview(hp):
                # p-major [key-in-chunk, kc-chunk, 160] with the ones columns
                # (64, 144) already embedded so gathers carry them along
                return kvb_hp[hp][0, KB:KVB].bitcast(FP8).rearrange(
                    "(p k4 c) -> p k4 c", k4=4, c=160)

            # ---------------- phase 1: QKV projections for this slice ----------------
            with (
                tc.tile_pool(name="ph1w", bufs=1) as ph1w,
                tc.tile_pool(name="ph1", bufs=3) as ph1,
                tc.tile_pool(name="psum1", bufs=2, space="PSUM") as psum1,
            ):
                # x^T chunk pairs (fp8, DoubleRow interleave [p, o, s]),
                # then weights in per-head-pair-group need-order so gathers
                # can fire early.
                xt_c = []
                for cp in range(4):
                    xt = ph1w.tile([128, 2, SL], FP8, tag=f"xt{cp}")
                    nc.sync.dma_start(xt[:], xT_d[256 * cp:256 * (cp + 1), :]
                                      .rearrange("(o p) s -> p o s", p=128))
                    xt_c.append(xt)
                wkp, wvp = {}, {}
                for g in range(4):
                    for cp in range(4):
                        wk = ph1w.tile([128, 2, 2, 128], FP8, tag=f"wk{g}_{cp}")
                        nc.sync.dma_start(
                            wk[:], wk_d[256 * cp:256 * (cp + 1), 256 * g:256 * (g + 1)]
                            .rearrange("(o p) (co q) -> p o co q", p=128, q=128))
                        wkp[(g, cp)] = wk
                    for cp in range(4):
                        wv = ph1w.tile([128, 2, 256], FP8, tag=f"wv{g}_{cp}")
                        nc.sync.dma_start(
                            wv[:], wv_d[256 * cp:256 * (cp + 1), 256 * g:256 * (g + 1)]
                            .rearrange("(o p) c -> p o c", p=128))
                        wvp[(g, cp)] = wv
                wq_c = []
                for cp in range(4):
                    wq = ph1w.tile([128, 2, 8, 128], FP8, tag=f"wq{cp}")
                    nc.sync.dma_start(wq[:], wq_d[256 * cp:256 * (cp + 1), :]
                                      .rearrange("(o p) (co q) -> p o co q", p=128, q=128))
                    wq_c.append(wq)

                def k_chunk(g, half):
                    co = 2 * g + half
                    ps = psum1.tile([128, SL], F32, tag="psk", name=f"psk{co}")
                    for cp in range(4):
                        nc.tensor.matmul(ps[:], wkp[(g, cp)][:, :, half, :], xt_c[cp][:],
                                         start=(cp == 0), stop=(cp == 3), perf_mode=DR)
                    kt_t = ph1.tile([128, SL], FP8, tag="kt", name=f"ktt{co}")
                    with nc.allow_low_precision("K in fp8"):
                        nc.vector.tensor_scalar_add(kt_t[:], ps[:], bk_sb[:, co:co + 1])
                    nc.gpsimd.dma_start(kvb_kview(co), kt_t[:])

                def v_pair(g):
                    ps = psum1.tile([128, 4, 256], F32, tag="psv", name=f"psv{g}")
                    for sc in range(4):
                        for cp in range(4):
                            nc.tensor.matmul(ps[:, sc, :],
                                             xt_c[cp][:, :, sc * 128:(sc + 1) * 128],
                                             wvp[(g, cp)][:],
                                             start=(cp == 0), stop=(cp == 3), perf_mode=DR)
                    v_t = ph1.tile([128, 4, 2, 160], FP8, tag="vt", name=f"vtt{g}")
                    nc.vector.memset(v_t[:, :, :, 64:65], 1.0)
                    nc.vector.memset(v_t[:, :, :, 144:145], 1.0)
                    with nc.allow_low_precision("V in fp8 for DoubleRow PV"):
                        for sc in range(4):
                            for half in range(2):
                                c0 = 256 * g + 128 * half
                                nc.vector.tensor_add(v_t[:, sc, half, 0:64],
                                                     ps[:, sc, 128 * half:128 * half + 64],
                                                     bv_bc[:, c0:c0 + 64])
                                nc.vector.tensor_add(v_t[:, sc, half, 80:144],
                                                     ps[:, sc, 128 * half + 64:128 * half + 128],
                                                     bv_bc[:, c0 + 64:c0 + 128])
                    for half in range(2):
                        nc.gpsimd.dma_start(kvb_vview(2 * g + half),
                                            v_t[:, :, half, :])

                def fire_cc(hp):
                    nc.gpsimd.collective_compute(
                        "AllGather", Alu.bypass, replica_groups=GROUPS,
                        ins=[kvb_hp[hp][:].opt()], outs=[kvg_hp[hp][:].opt()])

                for g in range(4):
                    k_chunk(g, 0)
                    k_chunk(g, 1)
                    v_pair(g)
                    fire_cc(2 * g)
                    fire_cc(2 * g + 1)

                # Q^T (local only; overlaps the gathers)
                for co in range(8):
                    ps = psum1.tile([128, SL], F32, tag="psk", name=f"psq{co}")
                    for cp in range(4):
                        nc.tensor.matmul(ps[:], wq_c[cp][:, :, co, :], xt_c[cp][:],
                                         start=(cp == 0), stop=(cp == 3), perf_mode=DR)
                    with nc.allow_low_precision("Q in fp8"):
                        nc.vector.tensor_scalar_add(qt_sb[:, co, :], ps[:], bq_sb[:, co:co + 1])

            # ---------------- phase 2: attention, one head pair at a time ----------------
            with (
                tc.tile_pool(name="kv", bufs=2) as kv,
                tc.tile_pool(name="expp", bufs=2) as expp,
                tc.tile_pool(name="small", bufs=3) as small,
                tc.tile_pool(name="ps_sc", bufs=3, space="PSUM") as ps_sc,
                tc.tile_pool(name="ps_o", bufs=1, space="PSUM") as ps_o,
            ):
                pending_norm = [None]

                def make_norm(hp, oAB):
                    def norm():
                        # softmax normalization: rows 0-63 = head dims,
                        # row 64 = sum(exp)
                        sAB = small.tile([1, 2, SL], F32, tag="sAB")
                        nc.scalar.copy(sAB[:], oAB[64:65, :, :])
                        rABf = small.tile([1, 2, SL], F32, tag="rABf")
                        nc.vector.reciprocal_approx_fast(out=rABf[:], in_=sAB[:])
                        rbS = small.tile([64, 2, SL], F32, tag="rbS")
                        nc.gpsimd.partition_broadcast(rbS[:], rABf[0:1, :, :])
                        with nc.allow_low_precision("attn out in fp8 for DR proj"):
                            nc.vector.scalar_tensor_tensor(
                                out=outT_sb[0:64, hp, :], in0=oAB[0:64, 0, :], scalar=1.0,
                                in1=rbS[:, 0, :], op0=Alu.bypass, op1=Alu.mult)
                            tmpB = small.tile([64, SL], FP8, tag="tmpB")
                            nc.vector.scalar_tensor_tensor(
                                out=tmpB[:], in0=oAB[0:64, 1, :], scalar=1.0,
                                in1=rbS[:, 1, :], op0=Alu.bypass, op1=Alu.mult)
                        nc.gpsimd.dma_start(outT_sb[64:128, hp, :], tmpB[:])
                    return norm

                for hp in range(8):
                    kth_t = kv.tile([128, 16, 128], FP8, tag="kth")
                    vh_t = vh_tiles[hp % 2]
                    # all 4 shards in one multi-dim DMA each (the shard
                    # stride in kvg is regular), 2 ring dispatches instead of 8
                    ksrc = kvg_hp[hp][:, 0:KB].bitcast(FP8).rearrange(
                        "j (p k4 m) -> p j k4 m", p=128, k4=4, m=128)
                    nc.sync.dma_start(
                        kth_t[:].rearrange("p (j k4) m -> p j k4 m", j=4), ksrc)
                    vsrc = kvg_hp[hp][:, KB:KVB].bitcast(FP8).rearrange(
                        "j (p jp o c) -> p j jp o c", p=128, jp=2, o=2, c=160)
                    nc.sync.dma_start(
                        vh_t[:].rearrange("p (j jp) o c -> p j jp o c", j=4), vsrc)
                    # phase-3 loads spread across the sync ring mid-phase-2
                    # so their bursts don't collide with any single gather
                    if hp == 2:
                        nc.sync.dma_start(xq_sb[:], xq_d.ap().rearrange("(qc p) d -> p qc d", p=128))
                    if hp == 4:
                        nc.sync.dma_start(wp_sb[:], wp_d.ap().rearrange("(hp p) (dh q) -> p hp dh q", p=128, q=512))
                    if hp == 5:
                        nc.sync.dma_start(gamma_bc[:], bcast_ap(gamma_d))
                        nc.sync.dma_start(beta_bc[:], bcast_ap(beta_d))

                    exp_t = expp.tile([128, 16, 2, 512], FP8, tag="exp")
                    oAB = ps_o.tile([65, 2, SL], F32, tag="oAB")

                    def pv_pair(j):
                        for h in range(2):
                            nc.tensor.matmul(oAB[:, h, :],
                                             vh_t[:, j, :, 80 * h:80 * h + 65],
                                             exp_t[:, 2 * j:2 * j + 2, h, :],
                                             start=(j == 0), stop=(j == 7),
                                             perf_mode=DR)

                    for kc in range(16):
                        ps = ps_sc.tile([128, 2, 512], F32, tag="sc")
                        # head A on PE rows 0-63, head B on rows 64-127 (row-tiled)
                        nc.tensor.matmul(ps[:, 0, :], kth_t[0:64, kc, :],
                                         qt_sb[0:64, hp, :], start=True, stop=True)
                        nc.tensor.matmul(ps[:, 1, :], kth_t[64:128, kc, :],
                                         qt_sb[64:128, hp, :], start=True, stop=True)
                        # per-head engine split: both exps run concurrently,
                        # halving the per-kc dependency latency
                        nc.scalar.activation(exp_t[:, kc, 0, :], ps[:, 0, :], Act.Exp,
                                             scale=0.125, bias=bsh_sb[:])
                        if kc >= 14:
                            # tail: scalar engine absorbs head B too (DVE is
                            # the busier engine across the hp boundary)
                            nc.scalar.activation(exp_t[:, kc, 1, :], ps[:, 1, :],
                                                 Act.Exp, scale=0.125, bias=bsh_sb[:])
                        else:
                            with nc.allow_low_precision("schraudolph fp8 exp"):
                                nc.vector.tensor_scalar(
                                    exp_t[:, kc, 1, :].bitcast(U8), ps[:, 1, :],
                                    EC1, EC2, Alu.mult, Alu.add)
                        # attn@V for pair j, 2.5 pairs behind the scores
                        if kc >= 5 and kc % 2 == 1:
                            pv_pair((kc - 5) // 2)
                    # kc loop emitted pairs 0..5; finish 6, 7
                    pv_pair(6)
                    pv_pair(7)
                    make_norm(hp, oAB)()

            # ---------------- phase 3: out-projection + residual + LayerNorm ----------------
            with (
                tc.tile_pool(name="ph3", bufs=3) as ph3,
                tc.tile_pool(name="ph3s", bufs=4) as ph3s,
                tc.tile_pool(name="psum3", bufs=4, space="PSUM") as psum3,
            ):
                for qc in range(4):
                    y_t = ph3.tile([128, D], F32, tag="y")
                    for dh in range(2):
                        ps = psum3.tile([128, 512], F32, tag="py")
                        # bp is folded into xq host-side, so the DR
                        # accumulation closes the group directly
                        for t in range(4):
                            nc.tensor.matmul(ps[:],
                                             outT_sb[:, 2 * t:2 * t + 2, qc * 128:(qc + 1) * 128],
                                             wp_sb[:, 2 * t:2 * t + 2, dh, :],
                                             start=(t == 0), stop=(t == 3), perf_mode=DR)
                        nc.vector.tensor_add(y_t[:, dh * 512:(dh + 1) * 512], ps[:],
                                             xq_sb[:, qc, dh * 512:(dh + 1) * 512])
                    # LayerNorm over D=1024
                    stats = ph3s.tile([128, 2, 6], F32, tag="stats")
                    nc.vector.bn_stats(stats[:, 0, :], y_t[:, 0:512])
                    nc.vector.bn_stats(stats[:, 1, :], y_t[:, 512:1024])
                    mv = ph3s.tile([128, 2], F32, tag="mv")
                    nc.vector.bn_aggr(mv[:], stats[:])
                    rstd = ph3s.tile([128, 1], F32, tag="rstd")
                    nc.scalar.activation(rstd[:], mv[:, 1:2], Act.Sqrt, bias=eps_sb[:])
                    nc.vector.reciprocal(rstd[:], rstd[:])
                    # y = ((y - mu) * gamma) * rstd + beta
                    nc.vector.scalar_tensor_tensor(
                        out=y_t[:], in0=y_t[:], scalar=mv[:, 0:1], in1=gamma_bc[:],
                        op0=Alu.subtract, op1=Alu.mult)
                    nc.vector.scalar_tensor_tensor(
                        out=y_t[:], in0=y_t[:], scalar=rstd[:], in1=beta_bc[:],
                        op0=Alu.mult, op1=Alu.add)
                    nc.sync.dma_start(y_d[qc * 128:(qc + 1) * 128, :], y_t[:])

    nc.compile()
    return nc


_PROGRAM = None


def _get_program():
    global _PROGRAM
    if _PROGRAM is None:
        _PROGRAM = build_program()
    return _PROGRAM


def kernel(**inputs):
    x = np.asarray(inputs["x"], np.float32)
    bf = ml_dtypes.bfloat16
    shared = {
        "wq": np.asarray(inputs["Wq"], np.float32).astype(ml_dtypes.float8_e4m3fn),
        "wk": np.asarray(inputs["Wk"], np.float32).astype(ml_dtypes.float8_e4m3fn),
        "wv": np.asarray(inputs["Wv"], np.float32).astype(ml_dtypes.float8_e4m3fn),
        "wp": np.asarray(inputs["Wp"], np.float32).astype(ml_dtypes.float8_e4m3fn),
        "bq": np.asarray(inputs["bq"], np.float32),
        "bk": np.asarray(inputs["bk"], np.float32),
        "bv": np.asarray(inputs["bv"], np.float32),
        "bp": np.asarray(inputs["bp"], np.float32).astype(bf),
        "gamma": np.asarray(inputs["gamma"], np.float32),
        "beta": np.asarray(inputs["beta"], np.float32),
    }
    in_maps = []
    for c in range(NCORES):
        b, i = c // 4, c % 4
        xs = np.ascontiguousarray(x[b, i * SL:(i + 1) * SL, :])
        m = dict(shared)
        m["xT"] = np.ascontiguousarray(xs.T).astype(ml_dtypes.float8_e4m3fn)
        # bp folded into the residual: y = (x + bp) + out @ Wp
        m["xq"] = xs + np.asarray(inputs["bp"], np.float32)
        in_maps.append(m)

    nc = _get_program()
    trace_dir = os.environ.get("BASS_KERNEL_TRACE_DIR")
    kwargs = {}
    if trace_dir:
        kwargs = {"trace": True, "tmpdir": trace_dir}
    res = run_bass_kernel_spmd(nc, in_maps, core_ids=list(range(NCORES)), **kwargs)

    out = np.empty((B, S, D), np.float32)
    for c in range(NCORES):
        b, i = c // 4, c % 4
        out[b, i * SL:(i + 1) * SL, :] = res.results[c]["y"]
    if trace_dir:
        kernel.last_exec_time_ns = res.exec_time_ns
        kernel.last_trace = res.instructions_and_trace
    return out
